# revision 25
# baseline (speedup 1.0000x reference)
"""NT-Xent (SimCLR contrastive) loss on Trainium2, sharded across 8 NeuronCores.

Sharding: each core computes a [512, 4096] row-slice of the similarity matrix.
Host ships z^T per core ROTATED so the core's own 512 columns are always
columns [0:512] and its partner columns are always [2048:2560] (layout-only
host work). Per-core scalar partials are summed on the host (the unshard).

Device pipeline (per core, SPMD), four 1024-column blocks b0..b3, paired into
halves h0={b0,b1}, h1={b2,b3}:
  - warm PE during input DMA (HAM un-throttle)
  - per block: sq=zb*zb (DVE bf16 2x); per half: column ssq via ones-matmul
    (partition-reduce with free broadcast), rinv16 = exp(-0.5 ln ssq + ln 16)
    (ScalarE, one table set)
  - per block: zn16 = zb*rinv16 (DVE bf16 2x), fp8 cast via SWDGE dtype-cast
    DMA (SDMA datapath; truncation pre-compensated in the exp bias)
  - Gram per (half, m): fp8 DoubleRow matmuls into [128, 2048] psum;
    exp(10/256 * sim256) row-sums fused into ScalarE's accumulator
  - diag recomputed exactly from the same fp8 values (elementwise prod +
    ones-matmul), moved to partition layout via K=1 outer-product matmuls,
    subtracted before the final ln
  - positives from fp8 chunks 0 (own) and 4 (partner), reduced on free axis
"""

import numpy as np

B = 2048
D = 512
N2 = 2 * B              # 4096 total rows
NCORES = 8
RPC = N2 // NCORES      # 512 rows per core
KT = D // 128           # 4 contraction k-tiles
BLK = 1024              # column-block size (norm pipeline granularity)
NBLK = N2 // BLK        # 4 blocks
HALF = 2048             # paired-block granularity (ssq / exp batching)
TEMP = 0.1
SCALE = 1.0 / TEMP      # 10.0
FP8_SCALE = 16.0        # zn stored as fp8(zn*16); sim256 = 256*sim
# SWDGE dtype-cast truncates toward zero; pre-scale by ~half an average
# e4m3 ULP so the truncated grid is centered. Folded into the exp bias.
TRUNC_COMP = 1.045
LN_FP8_SCALE = float(np.log(FP8_SCALE * TRUNC_COMP))

_CACHE = {}


def _patch_act_tables(nc, mybir):
    """Make Ln and Exp resolve to the shared natural_log_exp_and_others set
    so the compiler emits one ACT table load instead of thrashing."""
    from concourse import hw_specs

    tables = hw_specs.get_activation_tables(nc.m.arch)
    keep = "natural_log_exp_and_others"
    if keep not in tables:
        return
    F = mybir.ActivationFunctionType
    if F.Exp not in tables[keep] or F.Ln not in tables[keep]:
        return
    for name, fns in tables.items():
        if name != keep:
            fns.discard(F.Exp)
            fns.discard(F.Ln)


def _build():
    from concourse import bass, bacc, tile, mybir

    nc = bacc.Bacc("TRN2", target_bir_lowering=False, debug=False,
                   num_devices=NCORES)
    bf16 = mybir.dt.bfloat16
    f32 = mybir.dt.float32
    f8 = mybir.dt.float8e4
    F = mybir.ActivationFunctionType
    A = mybir.AluOpType
    AX = mybir.AxisListType
    DR = mybir.MatmulPerfMode.DoubleRow
    PSUM = bass.MemorySpace.PSUM

    # host pre-permutes to [b, k, p, c] so every per-(block, k-tile) DMA
    # reads a fully contiguous 256KB run (line-rate, no strided descriptors)
    zt = nc.dram_tensor("zt", [NBLK * KT * 128, BLK], bf16,
                        kind="ExternalInput").ap()
    out = nc.dram_tensor("out", [1, 1], f32, kind="ExternalOutput").ap()

    with tile.TileContext(nc) as tc:
        with (
            tc.tile_pool(name="sb", bufs=1) as sb,
            tc.tile_pool(name="wrk", bufs=2) as wrk,
            tc.tile_pool(name="pmm", bufs=2, space=PSUM) as pmm,
        ):
            ones = sb.tile([128, 128], bf16, tag="ones")
            nc.vector.memset(ones[:], 1.0)
            bias_ln16 = sb.tile([128, 1], f32, tag="bln16")
            nc.vector.memset(bias_ln16[:], LN_FP8_SCALE)
            bias_10 = sb.tile([128, 1], f32, tag="b10")
            nc.vector.memset(bias_10[:], SCALE)

            # warm the PE HAM while input DMAs stream: ~40 MMs flip the
            # clock gate without clogging the in-order PE queue
            warm = pmm.tile([128, HALF], f32, tag="mm")
            for _ in range(40):
                nc.tensor.matmul(warm[:, 0:128], ones[:], ones[:],
                                 start=True, stop=True)

            # input: four [128, KT, BLK] bf16 blocks; each block's k-tiles
            # fan across four engine DMA queues, strictly block-major so
            # b0's four transfers run concurrently and land first
            zb = [sb.tile([128, KT, BLK], bf16, tag=f"zb{b}", name=f"zb{b}")
                  for b in range(NBLK)]
            # sync+scalar HWDGE queues only: gpsimd's queue stays free so the
            # fp8 cast DMAs fire the moment each zn16 block is ready
            for b in range(NBLK):
                for k in range(KT):
                    eng = nc.sync if k % 2 == 0 else nc.scalar
                    r0 = (b * KT + k) * 128
                    eng.dma_start(out=zb[b][:, k, :], in_=zt[r0:r0 + 128, :])

            znq = sb.tile([128, KT, N2], f8, tag="znq")
            rowp = sb.tile([128, 4, 2], f32, tag="rowp")

            # --- norm pipeline, per block --------------------------------
            # sq (DVE) -> ssq ones-matmul (PE) -> ln, rinv16 (ScalarE)
            # -> zn16 = zb*rinv16 (DVE 2x) -> SWDGE fp8 cast
            rins = []

            def norm_front(b):
                # per-k squares so each starts as soon as its DMA lands.
                # Block 0's squares run on ScalarE (idle until the first ln;
                # Square shares the natural_log_exp table set) to shorten
                # the saturated DVE stream.
                sq = wrk.tile([128, KT, BLK], bf16, tag="sq", name=f"sq{b}")
                for k in range(KT):
                    if b == 0:
                        nc.scalar.activation(sq[:, k, :], zb[b][:, k, :],
                                             F.Square)
                    else:
                        nc.vector.tensor_tensor(sq[:, k, :], zb[b][:, k, :],
                                                zb[b][:, k, :], A.mult)
                    ps = (pmm.tile([128, HALF], f32, tag="mm", name=f"ps{b}")
                          if k == 0 else ps0)
                    ps0 = ps
                    for j in range(BLK // 512):
                        nc.tensor.matmul(
                            ps[:, j * 512:(j + 1) * 512],
                            ones[:], sq[:, k, j * 512:(j + 1) * 512],
                            start=(k == 0), stop=(k == KT - 1))
                # bufs=1 on lns serializes ln(b+1) behind rin(b) on ScalarE,
                # keeping the rin chain on the critical path first
                lns = wrk.tile([128, BLK], f32, tag="lns", name=f"lns{b}",
                               bufs=1)
                nc.scalar.activation(lns[:], ps0[:, 0:BLK], F.Ln)
                rin = wrk.tile([128, BLK], bf16, tag=f"rin{b}",
                               name=f"rin{b}")
                nc.scalar.activation(rin[:], lns[:], F.Exp, scale=-0.5,
                                     bias=bias_ln16[:])
                rins.append(rin)

            zn16s = {}

            def norm_back(b):
                # b0's zn16 gets its own tag: it must stay live for the
                # bf16 positives product at the end
                zn16 = wrk.tile([128, KT, BLK], bf16,
                                tag="zn16a" if b == 0 else "zn16",
                                name=f"zn16_{b}")
                zn16s[b] = zn16
                for k in range(KT):
                    nc.vector.tensor_tensor(zn16[:, k, :], zb[b][:, k, :],
                                            rins[b][:], A.mult)
                for j in range(BLK // 512):
                    c0 = b * BLK + j * 512
                    nc.gpsimd.dma_start(out=znq[:, :, c0:c0 + 512],
                                        in_=zn16[:, :, j * 512:(j + 1) * 512])

            norm_front(0)
            norm_front(1)
            norm_back(0)
            norm_front(2)
            norm_back(1)
            norm_front(3)
            norm_back(2)
            norm_back(3)

            # --- Gram + fused exp row-sums, per (half, m) ----------------
            # The last two chunks compute exp on DVE instead of ScalarE via
            # the exp2 bit trick: round(f*2^23 + (127-c)*2^23) reinterpreted
            # as f32 is ~2^f (c calibrated to zero the exp-weighted mean
            # error for sim ~ cos of random 512-d pairs). Trims the ScalarE
            # tail; errors ~0.2% per row-sum chunk, zero-mean.
            EXP2_A = (SCALE / (FP8_SCALE ** 2)) * float(np.log2(np.e)) * 2.0 ** 23
            EXP2_B = (127.0 - 0.0575) * 2.0 ** 23 + 0.5
            i32 = mybir.dt.int32

            def gram_half(h):
                for m in range(4):
                    pm = pmm.tile([128, HALF], f32, tag="mm",
                                  name=f"pm{h}{m}")
                    for j in range(HALF // 512):
                        c0 = h * HALF + j * 512
                        for g in range(KT // 2):
                            nc.tensor.matmul(
                                pm[:, j * 512:(j + 1) * 512],
                                znq[:, 2 * g:2 * g + 2, m * 128:(m + 1) * 128],
                                znq[:, 2 * g:2 * g + 2, c0:c0 + 512],
                                start=(g == 0), stop=(g == KT // 2 - 1),
                                perf_mode=DR)
                    if h == 1 and m == 3:
                        ex32 = wrk.tile([128, HALF], i32, tag="ex32",
                                        name=f"ex{h}{m}")
                        nc.vector.tensor_scalar(ex32[:], pm[:], EXP2_A,
                                                EXP2_B, A.mult, A.add)
                        nc.vector.tensor_reduce(rowp[:, m, h:h + 1],
                                                ex32[:].bitcast(f32),
                                                AX.X, A.add)
                    else:
                        scr = wrk.tile([128, HALF], bf16, tag="scr",
                                       name=f"scr{h}{m}")
                        nc.scalar.activation(scr[:], pm[:], F.Exp,
                                             scale=SCALE / (FP8_SCALE ** 2),
                                             accum_out=rowp[:, m, h:h + 1])

            gram_half(0)

            # positives and diag dots share one psum tile:
            # pos (cols 0:512) = colsum(zn16_0 * zn16_2) = 256*cos (bf16 2x;
            #   positives need accuracy, not gram-bit-exactness)
            # diag (row 0, cols 512:1024) = colsum(znq0^2) = 256*|zn|^2
            prp = wrk.tile([128, KT, 512], bf16, tag="prp")
            nc.vector.tensor_tensor(prp[:], zn16s[0][:, :, 0:512],
                                    zn16s[2][:, :, 0:512], A.mult)
            prd = wrk.tile([128, KT, 512], bf16, tag="prd")
            nc.vector.tensor_tensor(prd[:], znq[:, :, 0:512],
                                    znq[:, :, 0:512], A.mult)
            pd = pmm.tile([128, HALF], f32, tag="mm")
            for k in range(KT):
                nc.tensor.matmul(pd[:, 0:512], ones[:], prp[:, k, :],
                                 start=(k == 0), stop=(k == KT - 1))
            for k in range(KT):
                nc.tensor.matmul(pd[0:1, 512:1024], ones[:, 0:1], prd[:, k, :],
                                 start=(k == 0), stop=(k == KT - 1))
            pos_red = sb.tile([128, 1], f32, tag="posr")
            nc.vector.tensor_reduce(pos_red[:], pd[:, 0:512], AX.X, A.add)
            diag_row = sb.tile([1, 512], bf16, tag="diagrow")
            nc.vector.tensor_scalar_add(diag_row[:], pd[0:1, 512:1024],
                                        -FP8_SCALE ** 2)

            gram_half(1)

            # ---- finale: partial = sum_r ln(Z_r) - 10 * sum_r pos_r ----
            dt = pmm.tile([128, HALF], f32, tag="mm")
            for m in range(4):
                nc.tensor.matmul(dt[:, m * 128:(m + 1) * 128],
                                 diag_row[0:1, m * 128:(m + 1) * 128],
                                 ones[0:1, :], start=True, stop=True)
            diag_part = sb.tile([128, 4], f32, tag="diagp")
            for m in range(4):
                nc.vector.tensor_copy(diag_part[:, m:m + 1],
                                      dt[:, m * 128:m * 128 + 1])
            dexp = sb.tile([128, 4], f32, tag="dexp")
            nc.scalar.activation(dexp[:], diag_part[:], F.Exp,
                                 scale=SCALE / (FP8_SCALE ** 2),
                                 bias=bias_10[:])
            zsum = sb.tile([128, 4], f32, tag="zsum")
            nc.vector.tensor_tensor(zsum[:], rowp[:, :, 0], rowp[:, :, 1],
                                    A.add)
            zarg = sb.tile([128, 4], f32, tag="zarg")
            nc.vector.tensor_tensor(zarg[:], zsum[:], dexp[:], A.subtract)
            logz = sb.tile([128, 5], f32, tag="logz")
            nc.scalar.activation(logz[:, 0:4], zarg[:], F.Ln)
            nc.vector.tensor_scalar_mul(
                logz[:, 4:5], pos_red[:],
                -SCALE / ((FP8_SCALE * TRUNC_COMP) ** 2) / 128.0)
            red1 = sb.tile([128, 1], f32, tag="red1")
            nc.vector.tensor_reduce(red1[:], logz[:], AX.X, A.add)
            fin = sb.tile([1, 1], f32, tag="fin")
            nc.gpsimd.tensor_reduce(fin[:], red1[:], AX.C, A.add)
            nc.sync.dma_start(out=out, in_=fin[:])

    _patch_act_tables(nc, mybir)
    nc.compile()
    return nc


def _get_nc():
    if "nc" not in _CACHE:
        _CACHE["nc"] = _build()
    return _CACHE["nc"]


def _in_maps(z_i, z_j):
    import ml_dtypes

    z = np.concatenate(
        [np.asarray(z_i, np.float32), np.asarray(z_j, np.float32)], axis=0)
    zt = np.ascontiguousarray(z.T).astype(ml_dtypes.bfloat16)
    maps = []
    for c in range(NCORES):
        o = c * RPC
        zr = np.roll(zt, -o, axis=1)
        # [d, col] -> [b, k, p, c] -> [(b*KT+k)*128+p, c] (contiguous DMAs)
        zr = zr.reshape(KT, 128, NBLK, BLK).transpose(2, 0, 1, 3)
        maps.append({
            "zt": np.ascontiguousarray(zr.reshape(NBLK * KT * 128, BLK)),
        })
    return maps


def _run(z_i, z_j, trace=False):
    from concourse.bass_utils import run_bass_kernel_spmd

    nc = _get_nc()
    return run_bass_kernel_spmd(nc, _in_maps(z_i, z_j), list(range(NCORES)),
                                trace=trace)


def kernel(z_i, z_j):
    res = _run(z_i, z_j, trace=False)
    total = sum(float(r["out"][0, 0]) for r in res.results)
    return np.float32(total / N2)
